# revision 2
# baseline (speedup 1.0000x reference)
"""GQA attention with rotary embeddings (TransformerLens-style), distributed
over 8 TRN2 NeuronCores.

Sharding strategy (head-parallel attention, sequence-parallel W_O):
  - Core c owns query heads {2c, 2c+1} and kv head c//2.
  - Attention scores are computed TRANSPOSED (S^T = K Q^T, [k, q] layout)
    so the softmax'd pattern P^T is directly usable as the moving operand
    of Z^T = V^T P^T -- no PE transposes of P needed.  The softmax
    denominator r[q] = sum_k exp(s) comes from a ones-matmul accumulated
    alongside Z^T; 1/r is one fast custom-DVE op (reciprocal_approx_fast,
    ~18 bits) so the scalar engine runs ONLY Exp -> exactly one
    ACT_TABLE_LOAD for the whole kernel (the old Ln/Exp pair thrashed the
    table 17x).
  - Causal masking is multiplicative (0/1 bf16 mask on the exp'd pattern)
    instead of additive -1e9 on f32 psum.
  - Projections are streamed in 512-column seq panels (2MB DMAs); BOTH
    heads' attention pairs are interleaved into the panel loop (pair m
    depends only on panels <= m), so the PE never waits for a separate
    "phase B" and the first AllToAll fires ~50us earlier.
  - Z^T is exchanged with per-head AllToAlls; W_O weight loads are issued
    after the A2A triggers so they drain on the sync DMA ring during the
    collective's flight instead of competing with panel loads.  W_O is
    sequence-parallel per core: even heads (A2A#1) accumulate during
    A2A#2's flight, the output streams out per 512-column chunk.
"""
import os
import sys

if "/opt/trn_rl_repo" not in sys.path:
    sys.path.insert(0, "/opt/trn_rl_repo")

import ml_dtypes
import numpy as np

import concourse.bass as bass  # noqa: F401
import concourse.mybir as mybir
import concourse.tile as tile
from concourse import bacc
from concourse.bass_utils import run_bass_kernel_spmd

F32 = mybir.dt.float32
BF16 = mybir.dt.bfloat16
EXP = mybir.ActivationFunctionType.Exp

S = 2048
D = 2048
NH, NKV, DH = 16, 4, 128
ROTARY_BASE = 10000.0
NCORE = 8
HPC = NH // NCORE           # query heads per core
NP = 4                      # seq panels of 512
PW = 512                    # panel width
ISCALE = 1.0 / float(np.sqrt(DH))


def _build():
    nc = bacc.Bacc("TRN2", target_bir_lowering=False, debug=False,
                   num_devices=NCORE)

    # inputs pre-tiled host-side into [panel, partition, (Dchunk, seq)] so
    # each 2MB panel DMA is fully contiguous per partition (16KB lines)
    xq = nc.declare_dram_parameter("xq_t", [NP, 128, 16 * PW], BF16,
                                   isOutput=False)
    xk = nc.declare_dram_parameter("xk_t", [NP, 128, 16 * PW], BF16,
                                   isOutput=False)
    xv = nc.declare_dram_parameter("xv_t", [NP, 128, 16 * PW], BF16,
                                   isOutput=False)
    wq = nc.declare_dram_parameter("wq", [HPC, 128, D], BF16, isOutput=False)
    wk = nc.declare_dram_parameter("wk", [128, D], BF16, isOutput=False)
    wv = nc.declare_dram_parameter("wv", [128, D], BF16, isOutput=False)
    wo = nc.declare_dram_parameter("wo", [NH, DH, D], BF16, isOutput=False)
    cos_k = nc.declare_dram_parameter("cos_k", [DH, S], F32, isOutput=False)
    sin_k = nc.declare_dram_parameter("sin_k", [DH, S], F32, isOutput=False)
    ident_d = nc.declare_dram_parameter("ident", [128, 128], BF16,
                                        isOutput=False)
    ones_d = nc.declare_dram_parameter("ones", [128, 128], BF16,
                                       isOutput=False)
    mask01_d = nc.declare_dram_parameter("mask01", [128, 896], BF16,
                                         isOutput=False)
    out_ext = nc.declare_dram_parameter("out", [256, D], F32, isOutput=True)

    no_a2a = bool(os.environ.get("K_NO_A2A"))

    with tile.TileContext(nc) as tc:
        with tc.tile_pool(name="dram", bufs=1, space="DRAM") as dram, \
             tc.tile_pool(name="consts", bufs=1) as consts, \
             tc.tile_pool(name="qkv", bufs=1) as qkv, \
             tc.tile_pool(name="wos", bufs=16) as wos, \
             tc.tile_pool(name="ztrp", bufs=1) as ztrp:

            a2a_send = [dram.tile([NCORE, 128, 256], BF16, tag=f"send{h}",
                                  name=f"send{h}") for h in range(HPC)]
            a2a_recv = [dram.tile([NCORE, 128, 256], BF16, tag=f"recv{h}",
                                  name=f"recv{h}") for h in range(HPC)]

            ident = consts.tile([128, 128], BF16, tag="ident")
            ones = consts.tile([128, 128], BF16, tag="ones")
            mask01 = consts.tile([128, 896], BF16, tag="mask01")

            kt_sb = qkv.tile([128, S], BF16, tag="kt")
            qt_sb = [qkv.tile([128, S], BF16, name=f"qt{h}", tag=f"qt{h}")
                     for h in range(HPC)]
            v_sb = qkv.tile([128, S], BF16, tag="v")

            wo_t = [wos.tile([128, D], BF16, tag="wo", name=f"wo{hh}")
                    for hh in range(NH)]
            zt_r = [ztrp.tile([128, S], BF16, name=f"ztr{h}", tag=f"ztr{h}")
                    for h in range(HPC)]

            def attn_pair(h, m, ps, zp, rp, ptp, ztsp, rvp, sbufs=2):
                """Scores^T, softmax, Z^T and denominator for query columns
                [512m, 512m+512) of head h; sends normalized Z^T to the
                A2A buffer."""
                ngrp = 2 * m + 2          # groups of 2 k-tiles
                last_t = 4 * m + 3
                pt_g = [None] * ngrp

                def scores_grp(g):
                    s_ps = ps.tile([128, 1024], F32, tag="sps", bufs=sbufs,
                                   name=f"sps{h}_{m}_{g}")
                    for half in range(2):
                        t = 2 * g + half
                        nc.tensor.matmul(
                            s_ps[:, 512 * half:512 * (half + 1)],
                            kt_sb[:, 128 * t:128 * (t + 1)],
                            qt_sb[h][:, PW * m:PW * (m + 1)],
                            start=True, stop=True)
                    pt = ptp.tile([128, 1024], BF16, tag="pt",
                                  name=f"pt{h}_{m}_{g}")
                    nc.scalar.activation(pt[:], s_ps[:], EXP,
                                         bias=0.0, scale=ISCALE)
                    if g >= 2 * m:      # diagonal band: 0/1 mask
                        for half in range(2):
                            b = 2 * g + half - 4 * m
                            off = (3 - b) * 128
                            nc.vector.tensor_mul(
                                pt[:, 512 * half:512 * (half + 1)],
                                pt[:, 512 * half:512 * (half + 1)],
                                mask01[:, off:off + 512])
                    pt_g[g] = pt

                def zr_grp(g, z_ps, r_ps):
                    for half in range(2):
                        t = 2 * g + half
                        nc.tensor.matmul(
                            z_ps[:], v_sb[:, 128 * t:128 * (t + 1)],
                            pt_g[g][:, 512 * half:512 * (half + 1)],
                            start=(t == 0), stop=(t == last_t))
                        nc.tensor.matmul(
                            r_ps[:], ones[:],
                            pt_g[g][:, 512 * half:512 * (half + 1)],
                            start=(t == 0), stop=(t == last_t))

                z_ps = zp.tile([128, 512], F32, tag="z", name=f"z{h}_{m}")
                r_ps = rp.tile([128, 512], F32, tag="r", name=f"r{h}_{m}")
                # scores run 2 groups ahead of Z/R so exp latency is
                # hidden and pt tiles free progressively
                scores_grp(0)
                if ngrp > 1:
                    scores_grp(1)
                for g in range(ngrp):
                    if g + 2 < ngrp:
                        scores_grp(g + 2)
                    zr_grp(g, z_ps, r_ps)
                # 1/r in one custom-DVE op (~18 bits, plenty for a softmax
                # denominator; r >= exp(s_00) > 0 so no edge cases).  Keeps
                # Ln off the scalar engine -> no ACT table switches.
                rv = rvp.tile([128, 512], F32, tag="rv", name=f"rv{h}_{m}")
                nc.vector.reciprocal_approx_fast(rv[:], r_ps[:])
                zt = ztsp.tile([128, 512], BF16, tag="zt", name=f"zt{h}_{m}")
                nc.vector.tensor_mul(zt[:], z_ps[:], rv[:])
                nc.scalar.dma_start(
                    a2a_send[h][2 * m:2 * m + 2]
                    .rearrange("two p q -> p two q"),
                    zt[:].rearrange("p (two q) -> p two q", two=2))

            def do_a2a(h):
                if no_a2a:
                    nc.sync.dma_start(a2a_recv[h][:], a2a_send[h][:])
                else:
                    nc.gpsimd.collective_compute(
                        "AllToAll", mybir.AluOpType.bypass,
                        replica_groups=[list(range(NCORE))],
                        ins=[a2a_send[h].opt()],
                        outs=[a2a_recv[h].opt()])
                nc.scalar.dma_start(
                    zt_r[h][:].rearrange("p (i q) -> p i q", q=256),
                    a2a_recv[h][:].rearrange("i p q -> p i q"))

            # ------- phase A: projections + BOTH heads' attention --------
            with tc.tile_pool(name="trig", bufs=2) as trig, \
                 tc.tile_pool(name="wts", bufs=1) as wts, \
                 tc.tile_pool(name="xs", bufs=3) as xs, \
                 tc.tile_pool(name="vtp", bufs=2) as vtp, \
                 tc.tile_pool(name="rot", bufs=4) as rot, \
                 tc.tile_pool(name="ptA", bufs=4) as ptA, \
                 tc.tile_pool(name="ztsA", bufs=2) as ztsA, \
                 tc.tile_pool(name="rvA", bufs=2) as rvA, \
                 tc.tile_pool(name="psP", bufs=2, space="PSUM") as psP, \
                 tc.tile_pool(name="psA", bufs=1, space="PSUM") as psA, \
                 tc.tile_pool(name="psZ", bufs=1, space="PSUM") as psZ, \
                 tc.tile_pool(name="psR", bufs=1, space="PSUM") as psR:

                wk_sb = wts.tile([128, D], BF16, tag="wk")
                nc.sync.dma_start(wk_sb[:], wk[:])
                wq_sb = [wts.tile([128, D], BF16, name=f"wq{h}",
                                  tag=f"wq{h}") for h in range(HPC)]
                wv_sb = wts.tile([128, D], BF16, tag="wv")

                def panel_load(x_param, j, nm, split=1, dt=BF16):
                    xt = xs.tile([128, 16 * PW], dt, tag="xt", name=nm)
                    w = 16 * PW // split
                    for s in range(split):
                        nc.sync.dma_start(xt[:, w * s:w * (s + 1)],
                                          x_param[j][:, w * s:w * (s + 1)])
                    return xt

                def trig_load(j):
                    jr = slice(PW * j, PW * (j + 1))
                    ck = trig.tile([DH, PW], F32, tag="ck", name=f"ck{j}")
                    nc.sync.dma_start(ck[:], cos_k[:, jr])
                    sk = trig.tile([DH, PW], F32, tag="sk", name=f"sk{j}")
                    nc.sync.dma_start(sk[:], sin_k[:, jr])
                    return ck, sk

                def project(xt, w_sb, nm):
                    ps_t = psP.tile([128, PW], F32, tag="pp", name=nm)
                    for c in range(16):
                        nc.tensor.matmul(
                            ps_t[:], w_sb[:, 128 * c:128 * (c + 1)],
                            xt[:, PW * c:PW * (c + 1)],
                            start=(c == 0), stop=(c == 15))
                    return ps_t

                def rotary(ps_t, j, ck, sk, out_sb, nm):
                    jr = slice(PW * j, PW * (j + 1))
                    q2 = rot.tile([128, PW], F32, tag="rot", name=f"q2{nm}")
                    nc.vector.tensor_mul(q2[:], ps_t[:], ck[:])
                    sw = rot.tile([128, PW], F32, tag="rot", name=f"sw{nm}")
                    nc.vector.tensor_copy(sw[0:64, :], ps_t[64:128, :])
                    nc.vector.tensor_copy(sw[64:128, :], ps_t[0:64, :])
                    nc.vector.tensor_mul(sw[:], sw[:], sk[:])
                    nc.vector.tensor_add(out_sb[:, jr], q2[:], sw[:])

                for j in range(NP):
                    xt_k = panel_load(xk, j, f"xtk{j}", split=2 if j == 0
                                      else 1)
                    ck_j, sk_j = trig_load(j)
                    if j == 0:
                        for h in range(HPC):
                            nc.sync.dma_start(wq_sb[h][:], wq[h])
                    xt_q = panel_load(xq, j, f"xtq{j}", split=2 if j == 0
                                      else 1)
                    if j == 0:
                        nc.sync.dma_start(wv_sb[:], wv[:])
                    xt_v = panel_load(xv, j, f"xtv{j}")
                    if j == 0:
                        nc.sync.dma_start(ident[:], ident_d[:])
                        nc.sync.dma_start(ones[:], ones_d[:])
                        nc.sync.dma_start(mask01[:], mask01_d[:])
                    kp = project(xt_k, wk_sb, f"kp{j}")
                    rotary(kp, j, ck_j, sk_j, kt_sb, f"k{j}")
                    for h in range(HPC):
                        qp = project(xt_q, wq_sb[h], f"qp{h}_{j}")
                        rotary(qp, j, ck_j, sk_j, qt_sb[h], f"q{h}_{j}")
                    vp = project(xt_v, wv_sb, f"vp{j}")
                    vt_j = vtp.tile([128, PW], BF16, tag="vt", name=f"vt{j}")
                    nc.vector.tensor_copy(vt_j[:], vp[:])
                    tp = psP.tile([128, PW], BF16, tag="pp", name=f"tp{j}")
                    for i in range(4):
                        nc.tensor.transpose(
                            tp[:, 128 * i:128 * (i + 1)],
                            vt_j[:, 128 * i:128 * (i + 1)], ident[:])
                    nc.vector.tensor_copy(
                        v_sb[:, PW * j:PW * (j + 1)], tp[:])
                    # both heads' attention for query pair j interleaves
                    # with panel j+1's DMA/projection; z/r psum is shared
                    # (bufs=1) between the heads -- h1's first z matmul
                    # waits only for h0's zt mul to drain the bank
                    attn_pair(0, j, psA, psZ, psR, ptA, ztsA, rvA)
                    attn_pair(1, j, psA, psZ, psR, ptA, ztsA, rvA)

                do_a2a(0)
                do_a2a(1)
                # W_O loads issue on the sync ring AFTER the panel loads;
                # they drain during the A2A flight.  Even heads first --
                # phase C consumes them first.
                for hh in list(range(0, NH, 2)) + list(range(1, NH, 2)):
                    nc.sync.dma_start(wo_t[hh][:], wo[hh])

            # ------- phase C: W_O projection (seq-sharded) --------------
            with tc.tile_pool(name="ostp", bufs=2) as ostp, \
                 tc.tile_pool(name="psO", bufs=1, space="PSUM") as psO:
                o_ps = [psO.tile([128, D], F32, tag=f"o{s2}", name=f"o{s2}")
                        for s2 in range(2)]
                # even global heads arrive with A2A#1: their accumulation
                # overlaps A2A#2's flight
                for s2 in range(2):
                    for g in range(4):
                        for hh in range(0, NH, 2):
                            nc.tensor.matmul(
                                o_ps[s2][:, 512 * g:512 * (g + 1)],
                                zt_r[0][:, 256 * (hh // 2) + 128 * s2:
                                        256 * (hh // 2) + 128 * (s2 + 1)],
                                wo_t[hh][:, 512 * g:512 * (g + 1)],
                                start=(hh == 0), stop=False)
                for s2 in range(2):
                    for g in range(4):
                        for hh in range(1, NH, 2):
                            nc.tensor.matmul(
                                o_ps[s2][:, 512 * g:512 * (g + 1)],
                                zt_r[1][:, 256 * (hh // 2) + 128 * s2:
                                        256 * (hh // 2) + 128 * (s2 + 1)],
                                wo_t[hh][:, 512 * g:512 * (g + 1)],
                                start=False, stop=(hh == NH - 1))
                        ost = ostp.tile([128, 512], F32, tag="ost",
                                        name=f"ost{s2}_{g}")
                        nc.vector.tensor_copy(
                            ost[:], o_ps[s2][:, 512 * g:512 * (g + 1)])
                        nc.sync.dma_start(
                            out_ext[128 * s2:128 * (s2 + 1),
                                    512 * g:512 * (g + 1)], ost[:])

    nc.finalize()
    return nc


_NC_CACHE = None


def _get_nc():
    global _NC_CACHE
    if _NC_CACHE is None:
        _NC_CACHE = _build()
    return _NC_CACHE


def _rotary_tables():
    """cos/sin in transposed [dh, seq] layout with rotate-half sign folded
    into sin."""
    pos = np.arange(S, dtype=np.float64)
    dim = np.arange(DH // 2, dtype=np.float64)
    freq = ROTARY_BASE ** (dim / (DH / 2))
    freq = np.concatenate([freq, freq])
    ang = pos[None, :] / freq[:, None]
    cos_t = np.cos(ang)
    sin_t = np.sin(ang)
    sign = np.where(np.arange(DH) < DH // 2, -1.0, 1.0)[:, None]
    return (np.ascontiguousarray(cos_t.astype(np.float32)),
            np.ascontiguousarray((sin_t * sign).astype(np.float32)))


def _mask01():
    # mask01[kk, u] = 1 iff u >= kk + 384; band b of the diagonal uses
    # columns [(3-b)*128, (3-b)*128 + 512)
    kk = np.arange(128)[:, None]
    u = np.arange(896)[None, :]
    return np.ascontiguousarray(
        (u >= kk + 384).astype(ml_dtypes.bfloat16))


def _prep_w(w):
    # [D, DH] -> [128, 16*128]: partition = D%128, free = (D//128, dh)
    return np.ascontiguousarray(
        w.reshape(16, 128, 128).transpose(1, 0, 2).reshape(128, 2048))


def _prep_x(x):
    # [S, D] -> [NP, 128, 16*PW]: x_p[j, p, c*PW+s] = x[PW*j+s, 128*c+p],
    # so each panel is one fully-contiguous [128, 8192] DMA
    return np.ascontiguousarray(
        x.reshape(NP, PW, 16, 128).transpose(0, 3, 2, 1)
        .reshape(NP, 128, 16 * PW))


_last_in_maps = None


def kernel(query_input, key_input, value_input, W_Q, b_Q, W_K, b_K,
           W_V, b_V, W_O, b_O):
    nc = _get_nc()

    xq_t = _prep_x(np.asarray(query_input, np.float32)[0].astype(ml_dtypes.bfloat16))
    xk_t = _prep_x(np.asarray(key_input, np.float32)[0].astype(ml_dtypes.bfloat16))
    xv_t = _prep_x(np.asarray(value_input, np.float32)[0].astype(ml_dtypes.bfloat16))
    W_Q = np.asarray(W_Q, np.float32).astype(ml_dtypes.bfloat16)
    W_K = np.asarray(W_K, np.float32).astype(ml_dtypes.bfloat16)
    W_V = np.asarray(W_V, np.float32).astype(ml_dtypes.bfloat16)
    W_O = np.ascontiguousarray(np.asarray(W_O, np.float32).astype(ml_dtypes.bfloat16))

    cos_k, sin_k = _rotary_tables()
    mask01 = _mask01()
    ident = np.eye(128, dtype=ml_dtypes.bfloat16)
    ones = np.ones((128, 128), dtype=ml_dtypes.bfloat16)

    in_maps = []
    for c in range(NCORE):
        kv = c // 2
        in_maps.append({
            "xq_t": xq_t, "xk_t": xk_t, "xv_t": xv_t,
            "wq": np.stack([_prep_w(W_Q[2 * c + h]) for h in range(HPC)]),
            "wk": _prep_w(W_K[kv]),
            "wv": _prep_w(W_V[kv]),
            "wo": W_O,
            "cos_k": cos_k, "sin_k": sin_k,
            "ident": ident, "ones": ones, "mask01": mask01,
        })

    global _last_in_maps
    _last_in_maps = in_maps

    res = run_bass_kernel_spmd(nc, in_maps, core_ids=list(range(NCORE)))
    out = np.concatenate([res.results[c]["out"] for c in range(NCORE)],
                         axis=0)
    out = out + np.asarray(b_O, np.float32)[None, :]
    return out[None].astype(np.float32)


# revision 7
# speedup vs baseline: 1.3202x; 1.3202x over previous
"""GQA attention with rotary embeddings (TransformerLens-style), distributed
over 8 TRN2 NeuronCores.

Sharding strategy (head-parallel attention, sequence-parallel W_O):
  - Core c owns query heads {2c, 2c+1} and kv head c//2.
  - Attention scores are computed TRANSPOSED (S^T = K Q^T, [k, q] layout)
    so the softmax'd pattern P^T is directly usable as the moving operand
    of Z^T = V^T P^T -- no PE transposes of P needed.  The softmax
    denominator r[q] = sum_k exp(s) comes from a ones-matmul accumulated
    alongside Z^T; 1/r is one fast custom-DVE op (reciprocal_approx_fast,
    ~18 bits) so the scalar engine runs ONLY Exp -> exactly one
    ACT_TABLE_LOAD for the whole kernel (the old Ln/Exp pair thrashed the
    table 17x).
  - Causal masking is multiplicative (0/1 bf16 mask on the exp'd pattern)
    instead of additive -1e9 on f32 psum.
  - Projections are streamed in 512-column seq panels (2MB DMAs); BOTH
    heads' attention pairs are interleaved into the panel loop (pair m
    depends only on panels <= m), so the PE never waits for a separate
    "phase B" and the first AllToAll fires ~50us earlier.
  - Z^T is exchanged with per-head AllToAlls; W_O weight loads are issued
    after the A2A triggers so they drain on the sync DMA ring during the
    collective's flight instead of competing with panel loads.  W_O is
    sequence-parallel per core: even heads (A2A#1) accumulate during
    A2A#2's flight, the output streams out per 512-column chunk.
"""
import os
import sys

if "/opt/trn_rl_repo" not in sys.path:
    sys.path.insert(0, "/opt/trn_rl_repo")

import ml_dtypes
import numpy as np

import concourse.bass as bass  # noqa: F401
import concourse.mybir as mybir
import concourse.tile as tile
from concourse import bacc
from concourse.bass_utils import run_bass_kernel_spmd

F32 = mybir.dt.float32
BF16 = mybir.dt.bfloat16
EXP = mybir.ActivationFunctionType.Exp

S = 2048
D = 2048
NH, NKV, DH = 16, 4, 128
ROTARY_BASE = 10000.0
NCORE = 8
HPC = NH // NCORE           # query heads per core
NP = 4                      # seq panels of 512
PW = 512                    # panel width
ISCALE = 1.0 / float(np.sqrt(DH))


def _build():
    nc = bacc.Bacc("TRN2", target_bir_lowering=False, debug=False,
                   num_devices=NCORE)

    # inputs pre-tiled host-side into [panel, partition, (Dchunk, seq)] so
    # each 2MB panel DMA is fully contiguous per partition (16KB lines)
    xq = nc.declare_dram_parameter("xq_t", [NP, 128, 16 * PW], BF16,
                                   isOutput=False)
    xk = nc.declare_dram_parameter("xk_t", [NP, 128, 16 * PW], BF16,
                                   isOutput=False)
    xv = nc.declare_dram_parameter("xv_t", [NP, 128, 16 * PW], BF16,
                                   isOutput=False)
    wq = nc.declare_dram_parameter("wq", [HPC, 128, D], BF16, isOutput=False)
    wk = nc.declare_dram_parameter("wk", [128, D], BF16, isOutput=False)
    wv = nc.declare_dram_parameter("wv", [128, D], BF16, isOutput=False)
    wo = nc.declare_dram_parameter("wo", [NH, DH, D], BF16, isOutput=False)
    cos_k = nc.declare_dram_parameter("cos_k", [DH, S], F32, isOutput=False)
    sin_k = nc.declare_dram_parameter("sin_k", [DH, S], F32, isOutput=False)
    ident_d = nc.declare_dram_parameter("ident", [128, 128], BF16,
                                        isOutput=False)
    ones_d = nc.declare_dram_parameter("ones", [128, 128], BF16,
                                       isOutput=False)
    mask01_d = nc.declare_dram_parameter("mask01", [128, 896], BF16,
                                         isOutput=False)
    # bf16 output: halves the tail DMA; host casts back to f32 (adds
    # <=0.4% relative rounding, well inside the 2e-2 gate)
    out_ext = nc.declare_dram_parameter("out", [256, D], BF16, isOutput=True)

    no_a2a = bool(os.environ.get("K_NO_A2A"))

    with tile.TileContext(nc) as tc:
        with tc.tile_pool(name="dram", bufs=1, space="DRAM") as dram, \
             tc.tile_pool(name="consts", bufs=1) as consts, \
             tc.tile_pool(name="qkv", bufs=1) as qkv, \
             tc.tile_pool(name="wos", bufs=16) as wos, \
             tc.tile_pool(name="ztrp", bufs=1) as ztrp:

            a2a_send = [dram.tile([NCORE, 128, 256], BF16, tag=f"send{h}",
                                  name=f"send{h}") for h in range(HPC)]
            a2a_recv = [dram.tile([NCORE, 128, 256], BF16, tag=f"recv{h}",
                                  name=f"recv{h}") for h in range(HPC)]

            ident = consts.tile([128, 128], BF16, tag="ident")
            ones = consts.tile([128, 128], BF16, tag="ones")
            mask01 = consts.tile([128, 896], BF16, tag="mask01")

            kt_sb = qkv.tile([128, S], BF16, tag="kt")
            qt_sb = [qkv.tile([128, S], BF16, name=f"qt{h}", tag=f"qt{h}")
                     for h in range(HPC)]
            v_sb = qkv.tile([128, S], BF16, tag="v")

            wo_t = [wos.tile([128, D], BF16, tag="wo", name=f"wo{hh}")
                    for hh in range(NH)]
            # per-source-rank chunks: each is one contiguous 64KB read of
            # a2a_recv[h][c], and the W_O matmuls for head 2c+h depend only
            # on their own chunk -- accumulation starts as chunks land
            zt_r = [[ztrp.tile([128, 256], BF16, name=f"ztr{h}_{c}",
                               tag=f"ztr{h}_{c}") for c in range(NCORE)]
                    for h in range(HPC)]

            def attn_pair(h, m, ps, zp, rp, ptp, ztsp, rvp, sbufs=2):
                """Scores^T, softmax, Z^T and denominator for query columns
                [512m, 512m+512) of head h; sends normalized Z^T to the
                A2A buffer."""
                ngrp = 2 * m + 2          # groups of 2 k-tiles
                last_t = 4 * m + 3
                pt_g = [None] * ngrp

                def scores_grp(g):
                    s_ps = ps.tile([128, 1024], F32, tag="sps", bufs=sbufs,
                                   name=f"sps{h}_{m}_{g}")
                    for half in range(2):
                        t = 2 * g + half
                        nc.tensor.matmul(
                            s_ps[:, 512 * half:512 * (half + 1)],
                            kt_sb[:, 128 * t:128 * (t + 1)],
                            qt_sb[h][:, PW * m:PW * (m + 1)],
                            start=True, stop=True)
                    pt = ptp.tile([128, 1024], BF16, tag="pt",
                                  name=f"pt{h}_{m}_{g}")
                    nc.scalar.activation(pt[:], s_ps[:], EXP,
                                         bias=0.0, scale=ISCALE)
                    if g >= 2 * m:      # diagonal band: 0/1 mask
                        for half in range(2):
                            b = 2 * g + half - 4 * m
                            off = (3 - b) * 128
                            nc.vector.tensor_mul(
                                pt[:, 512 * half:512 * (half + 1)],
                                pt[:, 512 * half:512 * (half + 1)],
                                mask01[:, off:off + 512])
                    pt_g[g] = pt

                def zr_grp(g, z_ps, r_ps):
                    for half in range(2):
                        t = 2 * g + half
                        nc.tensor.matmul(
                            z_ps[:], v_sb[:, 128 * t:128 * (t + 1)],
                            pt_g[g][:, 512 * half:512 * (half + 1)],
                            start=(t == 0), stop=(t == last_t))
                        nc.tensor.matmul(
                            r_ps[:], ones[:],
                            pt_g[g][:, 512 * half:512 * (half + 1)],
                            start=(t == 0), stop=(t == last_t))

                z_ps = zp.tile([128, 512], F32, tag="z", name=f"z{h}_{m}")
                r_ps = rp.tile([128, 512], F32, tag="r", name=f"r{h}_{m}")
                # scores run 2 groups ahead of Z/R so exp latency is
                # hidden and pt tiles free progressively
                scores_grp(0)
                if ngrp > 1:
                    scores_grp(1)
                for g in range(ngrp):
                    if g + 2 < ngrp:
                        scores_grp(g + 2)
                    zr_grp(g, z_ps, r_ps)
                # 1/r in one custom-DVE op (~18 bits, plenty for a softmax
                # denominator; r >= exp(s_00) > 0 so no edge cases).  Keeps
                # Ln off the scalar engine -> no ACT table switches.
                rv = rvp.tile([128, 512], F32, tag="rv", name=f"rv{h}_{m}")
                nc.vector.reciprocal_approx_fast(rv[:], r_ps[:])
                zt = ztsp.tile([128, 512], BF16, tag="zt", name=f"zt{h}_{m}")
                nc.vector.tensor_mul(zt[:], z_ps[:], rv[:])
                nc.scalar.dma_start(
                    a2a_send[h][2 * m:2 * m + 2]
                    .rearrange("two p q -> p two q"),
                    zt[:].rearrange("p (two q) -> p two q", two=2))

            def do_a2a(h):
                if no_a2a:
                    nc.sync.dma_start(a2a_recv[h][:], a2a_send[h][:])
                else:
                    nc.gpsimd.collective_compute(
                        "AllToAll", mybir.AluOpType.bypass,
                        replica_groups=[list(range(NCORE))],
                        ins=[a2a_send[h].opt()],
                        outs=[a2a_recv[h].opt()])
                for c in range(NCORE):
                    nc.scalar.dma_start(zt_r[h][c][:], a2a_recv[h][c])

            # ------- phase A: projections + BOTH heads' attention --------
            with tc.tile_pool(name="trig", bufs=2) as trig, \
                 tc.tile_pool(name="wts", bufs=1) as wts, \
                 tc.tile_pool(name="xs", bufs=3) as xs, \
                 tc.tile_pool(name="vtp", bufs=2) as vtp, \
                 tc.tile_pool(name="rot", bufs=4) as rot, \
                 tc.tile_pool(name="ptA", bufs=4) as ptA, \
                 tc.tile_pool(name="ztsA", bufs=2) as ztsA, \
                 tc.tile_pool(name="rvA", bufs=2) as rvA, \
                 tc.tile_pool(name="psP", bufs=2, space="PSUM") as psP, \
                 tc.tile_pool(name="psA", bufs=1, space="PSUM") as psA, \
                 tc.tile_pool(name="psZ", bufs=1, space="PSUM") as psZ, \
                 tc.tile_pool(name="psR", bufs=1, space="PSUM") as psR:

                wk_sb = wts.tile([128, D], BF16, tag="wk")
                nc.sync.dma_start(wk_sb[:], wk[:])
                wq_sb = [wts.tile([128, D], BF16, name=f"wq{h}",
                                  tag=f"wq{h}") for h in range(HPC)]
                wv_sb = wts.tile([128, D], BF16, tag="wv")

                def panel_load(x_param, j, nm, split=1, dt=BF16):
                    xt = xs.tile([128, 16 * PW], dt, tag="xt", name=nm)
                    w = 16 * PW // split
                    for s in range(split):
                        nc.sync.dma_start(xt[:, w * s:w * (s + 1)],
                                          x_param[j][:, w * s:w * (s + 1)])
                    return xt

                def trig_load(j):
                    jr = slice(PW * j, PW * (j + 1))
                    ck = trig.tile([DH, PW], F32, tag="ck", name=f"ck{j}")
                    nc.sync.dma_start(ck[:], cos_k[:, jr])
                    sk = trig.tile([DH, PW], F32, tag="sk", name=f"sk{j}")
                    nc.sync.dma_start(sk[:], sin_k[:, jr])
                    return ck, sk

                def project(xt, w_sb, nm):
                    ps_t = psP.tile([128, PW], F32, tag="pp", name=nm)
                    for c in range(16):
                        nc.tensor.matmul(
                            ps_t[:], w_sb[:, 128 * c:128 * (c + 1)],
                            xt[:, PW * c:PW * (c + 1)],
                            start=(c == 0), stop=(c == 15))
                    return ps_t

                def rotary(ps_t, j, ck, sk, out_sb, nm):
                    jr = slice(PW * j, PW * (j + 1))
                    q2 = rot.tile([128, PW], F32, tag="rot", name=f"q2{nm}")
                    nc.vector.tensor_mul(q2[:], ps_t[:], ck[:])
                    sw = rot.tile([128, PW], F32, tag="rot", name=f"sw{nm}")
                    nc.vector.tensor_copy(sw[0:64, :], ps_t[64:128, :])
                    nc.vector.tensor_copy(sw[64:128, :], ps_t[0:64, :])
                    nc.vector.tensor_mul(sw[:], sw[:], sk[:])
                    nc.vector.tensor_add(out_sb[:, jr], q2[:], sw[:])

                for j in range(NP):
                    xt_k = panel_load(xk, j, f"xtk{j}", split=2 if j == 0
                                      else 1)
                    ck_j, sk_j = trig_load(j)
                    if j == 0:
                        for h in range(HPC):
                            nc.sync.dma_start(wq_sb[h][:], wq[h])
                    xt_q = panel_load(xq, j, f"xtq{j}", split=2 if j == 0
                                      else 1)
                    if j == 0:
                        nc.sync.dma_start(wv_sb[:], wv[:])
                    xt_v = panel_load(xv, j, f"xtv{j}")
                    if j == 0:
                        nc.sync.dma_start(ident[:], ident_d[:])
                        nc.sync.dma_start(ones[:], ones_d[:])
                        nc.sync.dma_start(mask01[:], mask01_d[:])
                    kp = project(xt_k, wk_sb, f"kp{j}")
                    rotary(kp, j, ck_j, sk_j, kt_sb, f"k{j}")
                    for h in range(HPC):
                        qp = project(xt_q, wq_sb[h], f"qp{h}_{j}")
                        rotary(qp, j, ck_j, sk_j, qt_sb[h], f"q{h}_{j}")
                    vp = project(xt_v, wv_sb, f"vp{j}")
                    vt_j = vtp.tile([128, PW], BF16, tag="vt", name=f"vt{j}")
                    nc.vector.tensor_copy(vt_j[:], vp[:])
                    tp = psP.tile([128, PW], BF16, tag="pp", name=f"tp{j}")
                    for i in range(4):
                        nc.tensor.transpose(
                            tp[:, 128 * i:128 * (i + 1)],
                            vt_j[:, 128 * i:128 * (i + 1)], ident[:])
                    nc.vector.tensor_copy(
                        v_sb[:, PW * j:PW * (j + 1)], tp[:])
                    # both heads' attention for query pair j interleaves
                    # with panel j+1's DMA/projection; z/r psum is shared
                    # (bufs=1) between the heads -- h1's first z matmul
                    # waits only for h0's zt mul to drain the bank
                    attn_pair(0, j, psA, psZ, psR, ptA, ztsA, rvA)
                    attn_pair(1, j, psA, psZ, psR, ptA, ztsA, rvA)

                do_a2a(0)
                do_a2a(1)
                # W_O loads issue on the sync ring AFTER the panel loads;
                # they drain during the A2A flight.  Even heads first --
                # phase C consumes them first.
                for hh in list(range(0, NH, 2)) + list(range(1, NH, 2)):
                    nc.sync.dma_start(wo_t[hh][:], wo[hh])

            # ------- phase C: W_O projection (seq-sharded) --------------
            with tc.tile_pool(name="ostp", bufs=2) as ostp, \
                 tc.tile_pool(name="psO", bufs=1, space="PSUM") as psO:
                o_ps = [psO.tile([128, D], F32, tag=f"o{s2}", name=f"o{s2}")
                        for s2 in range(2)]
                # even global heads arrive with A2A#1: their accumulation
                # overlaps A2A#2's flight
                for s2 in range(2):
                    for g in range(4):
                        for hh in range(0, NH, 2):
                            nc.tensor.matmul(
                                o_ps[s2][:, 512 * g:512 * (g + 1)],
                                zt_r[0][hh // 2][:, 128 * s2:128 * (s2 + 1)],
                                wo_t[hh][:, 512 * g:512 * (g + 1)],
                                start=(hh == 0), stop=False)
                for s2 in range(2):
                    for g in range(4):
                        for hh in range(1, NH, 2):
                            nc.tensor.matmul(
                                o_ps[s2][:, 512 * g:512 * (g + 1)],
                                zt_r[1][hh // 2][:, 128 * s2:128 * (s2 + 1)],
                                wo_t[hh][:, 512 * g:512 * (g + 1)],
                                start=False, stop=(hh == NH - 1))
                        ost = ostp.tile([128, 512], BF16, tag="ost",
                                        name=f"ost{s2}_{g}")
                        nc.vector.tensor_copy(
                            ost[:], o_ps[s2][:, 512 * g:512 * (g + 1)])
                        nc.sync.dma_start(
                            out_ext[128 * s2:128 * (s2 + 1),
                                    512 * g:512 * (g + 1)], ost[:])

    nc.finalize()
    return nc


_NC_CACHE = None


def _get_nc():
    global _NC_CACHE
    if _NC_CACHE is None:
        _NC_CACHE = _build()
    return _NC_CACHE


def _rotary_tables():
    """cos/sin in transposed [dh, seq] layout with rotate-half sign folded
    into sin."""
    pos = np.arange(S, dtype=np.float64)
    dim = np.arange(DH // 2, dtype=np.float64)
    freq = ROTARY_BASE ** (dim / (DH / 2))
    freq = np.concatenate([freq, freq])
    ang = pos[None, :] / freq[:, None]
    cos_t = np.cos(ang)
    sin_t = np.sin(ang)
    sign = np.where(np.arange(DH) < DH // 2, -1.0, 1.0)[:, None]
    return (np.ascontiguousarray(cos_t.astype(np.float32)),
            np.ascontiguousarray((sin_t * sign).astype(np.float32)))


def _mask01():
    # mask01[kk, u] = 1 iff u >= kk + 384; band b of the diagonal uses
    # columns [(3-b)*128, (3-b)*128 + 512)
    kk = np.arange(128)[:, None]
    u = np.arange(896)[None, :]
    return np.ascontiguousarray(
        (u >= kk + 384).astype(ml_dtypes.bfloat16))


def _prep_w(w):
    # [D, DH] -> [128, 16*128]: partition = D%128, free = (D//128, dh)
    return np.ascontiguousarray(
        w.reshape(16, 128, 128).transpose(1, 0, 2).reshape(128, 2048))


def _prep_x(x):
    # [S, D] -> [NP, 128, 16*PW]: x_p[j, p, c*PW+s] = x[PW*j+s, 128*c+p],
    # so each panel is one fully-contiguous [128, 8192] DMA
    return np.ascontiguousarray(
        x.reshape(NP, PW, 16, 128).transpose(0, 3, 2, 1)
        .reshape(NP, 128, 16 * PW))


_last_in_maps = None


def kernel(query_input, key_input, value_input, W_Q, b_Q, W_K, b_K,
           W_V, b_V, W_O, b_O):
    nc = _get_nc()

    xq_t = _prep_x(np.asarray(query_input, np.float32)[0].astype(ml_dtypes.bfloat16))
    xk_t = _prep_x(np.asarray(key_input, np.float32)[0].astype(ml_dtypes.bfloat16))
    xv_t = _prep_x(np.asarray(value_input, np.float32)[0].astype(ml_dtypes.bfloat16))
    W_Q = np.asarray(W_Q, np.float32).astype(ml_dtypes.bfloat16)
    W_K = np.asarray(W_K, np.float32).astype(ml_dtypes.bfloat16)
    W_V = np.asarray(W_V, np.float32).astype(ml_dtypes.bfloat16)
    W_O = np.ascontiguousarray(np.asarray(W_O, np.float32).astype(ml_dtypes.bfloat16))

    cos_k, sin_k = _rotary_tables()
    mask01 = _mask01()
    ident = np.eye(128, dtype=ml_dtypes.bfloat16)
    ones = np.ones((128, 128), dtype=ml_dtypes.bfloat16)

    in_maps = []
    for c in range(NCORE):
        kv = c // 2
        in_maps.append({
            "xq_t": xq_t, "xk_t": xk_t, "xv_t": xv_t,
            "wq": np.stack([_prep_w(W_Q[2 * c + h]) for h in range(HPC)]),
            "wk": _prep_w(W_K[kv]),
            "wv": _prep_w(W_V[kv]),
            "wo": W_O,
            "cos_k": cos_k, "sin_k": sin_k,
            "ident": ident, "ones": ones, "mask01": mask01,
        })

    global _last_in_maps
    _last_in_maps = in_maps

    res = run_bass_kernel_spmd(nc, in_maps, core_ids=list(range(NCORE)))
    out = np.concatenate([np.asarray(res.results[c]["out"])
                          .astype(np.float32) for c in range(NCORE)], axis=0)
    out = out + np.asarray(b_O, np.float32)[None, :]
    return out[None].astype(np.float32)
